# revision 2
# baseline (speedup 1.0000x reference)
"""Trainium2 Bass kernel for ClipPairWiseLossAll.

loss = sum_{i<j} || relu(r_i - r_j) ||_2   with r = repr[GT], M=512, N=768.

Strategy (8 NeuronCores, SPMD, single NEFF):
  * Host: gather r = repr[GT], transpose -> rT [N=768, M=512], cast bf16.
  * The pair space is indexed (t, s) with t < s; the "s" axis (512 values) is
    split across the 8 cores with a folded assignment (core c gets s in
    {16k+c, 16k+15-c}) so every core covers exactly 16352 pairs.
  * On device, rT lives in SBUF as 6 chunks of [128 n-partitions, 512 i-free].
    For each s: E[n, t] = relu(r[t,n] - r[s,n]) is ONE tensor_scalar per chunk
    (per-partition scalar = column rT[:, s]; runs in DVE 4x bf16 mode).
    E2 = E*E (DVE scalar_tensor_tensor or ACT Square, split to balance).
    A ones-column matmul contracts E2 over the n-partitions into PSUM row m_s
    (one-hot lhsT), accumulating the 6 chunks: psum[m_s, t] = sum_n E^2.
  * Valid-t ranges are rounded up to L_k = 16(k+1) (identical instruction
    shapes on every core -> one NEFF); a per-core 0/1 mask kills the invalid
    columns, then ACT computes sqrt with a fused row-sum. Host adds 8x64
    partial sums.
"""

import numpy as np

M = 512
N = 768
P = 128
NCH = N // P  # 6
NCORES = 8
NS = 64  # s values per core


def _s_list(c):
    out = []
    for k in range(32):
        out.append(16 * k + c)
        out.append(16 * k + 15 - c)
    return out


_PROG = {}


def _build_program():
    if "nc" in _PROG:
        return _PROG["nc"]

    from contextlib import ExitStack

    import concourse.bacc as bacc
    import concourse.tile as tile
    from concourse import mybir

    AOT = mybir.AluOpType
    AFT = mybir.ActivationFunctionType
    bf16 = mybir.dt.bfloat16
    f32 = mybir.dt.float32

    nc = bacc.Bacc(
        "TRN2",
        target_bir_lowering=False,
        debug=False,
        enable_asserts=False,
        num_devices=NCORES,
    )

    rt_d = nc.dram_tensor("rt", [N, M], bf16, kind="ExternalInput")
    sc_d = nc.dram_tensor("sc", [N, NS], f32, kind="ExternalInput")
    mk_d = nc.dram_tensor("mk", [NS, M], f32, kind="ExternalInput")
    out_d = nc.dram_tensor("out", [NS, 1], f32, kind="ExternalOutput")

    with ExitStack() as ctx:
        tc = ctx.enter_context(tile.TileContext(nc))
        singles = ctx.enter_context(tc.tile_pool(name="singles", bufs=1))
        epool = ctx.enter_context(tc.tile_pool(name="e", bufs=4))
        e2pool = ctx.enter_context(tc.tile_pool(name="e2", bufs=4))
        pspool = ctx.enter_context(tc.tile_pool(name="ps", bufs=1, space="PSUM"))

        rt_sb = singles.tile([P, NCH, M], bf16)
        nc.sync.dma_start(out=rt_sb, in_=rt_d.ap().rearrange("(c p) i -> p c i", p=P))
        sc_sb = singles.tile([P, NCH, NS], f32)
        nc.sync.dma_start(out=sc_sb, in_=sc_d.ap().rearrange("(c p) s -> p c s", p=P))
        mk_sb = singles.tile([NS, M], f32)
        nc.sync.dma_start(out=mk_sb, in_=mk_d.ap())

        # one-hot bank: column 63 is ones; slice [63-m : 127-m] puts the ones
        # column at position m of a [128, 64] lhsT.
        oh = singles.tile([P, 2 * NS - 1], bf16)
        nc.vector.memset(oh, 0.0)
        nc.vector.memset(oh[:, NS - 1 : NS], 1.0)

        ps = pspool.tile([NS, M], f32)
        nc.vector.memset(ps, 0.0)

        for k in range(32):
            L = 16 * (k + 1)
            for slot in range(2):
                m = 2 * k + slot
                e_t = epool.tile([P, NCH, M], bf16, tag="e")
                for c in range(NCH):
                    nc.vector.tensor_scalar(
                        out=e_t[:, c, 0:L],
                        in0=rt_sb[:, c, 0:L],
                        scalar1=sc_sb[:, c, m : m + 1],
                        scalar2=0.0,
                        op0=AOT.subtract,
                        op1=AOT.max,
                    )
                e2_t = e2pool.tile([P, NCH, M], bf16, tag="e2")
                if slot == 1:
                    nc.scalar.activation(
                        out=e2_t[:, :, 0:L],
                        in_=e_t[:, :, 0:L],
                        func=AFT.Square,
                    )
                else:
                    nc.vector.scalar_tensor_tensor(
                        out=e2_t[:, :, 0:L],
                        in0=e_t[:, :, 0:L],
                        scalar=0.0,
                        in1=e_t[:, :, 0:L],
                        op0=AOT.bypass,
                        op1=AOT.mult,
                    )
                for c in range(NCH):
                    nc.tensor.matmul(
                        ps[:, 0:L],
                        oh[:, NS - 1 - m : 2 * NS - 1 - m],
                        e2_t[:, c, 0:L],
                        start=False,
                        stop=False,
                        skip_group_check=True,
                    )

        masked = singles.tile([NS, M], f32)
        nc.vector.tensor_mul(masked, ps[:, :], mk_sb)
        sqrt_t = singles.tile([NS, M], bf16)
        res = singles.tile([NS, 1], f32)
        nc.scalar.activation(out=sqrt_t, in_=masked, func=AFT.Sqrt, accum_out=res)
        nc.sync.dma_start(out=out_d.ap(), in_=res)

    nc.compile()
    _PROG["nc"] = nc
    return nc


def _in_maps(repr_np, GT_np):
    import ml_dtypes

    r = np.asarray(repr_np, dtype=np.float32)[np.asarray(GT_np).astype(np.int64)]
    rT = np.ascontiguousarray(r.T)  # [N, M] f32
    rT_bf = rT.astype(ml_dtypes.bfloat16)

    maps = []
    t_idx = np.arange(M)[None, :]
    for c in range(NCORES):
        s = np.array(_s_list(c))
        sc = np.ascontiguousarray(rT_bf[:, s].astype(np.float32))  # [N, 64]
        mk = (t_idx < s[:, None]).astype(np.float32)  # [64, M]
        maps.append({"rt": rT_bf, "sc": sc, "mk": mk})
    return maps


def run_device(repr_np, GT_np, trace=False, trace_cores=None):
    """Run the bass kernel on 8 cores; returns (total, BassKernelResults)."""
    from concourse.bass_utils import run_bass_kernel_spmd

    nc = _build_program()
    maps = _in_maps(repr_np, GT_np)
    res = run_bass_kernel_spmd(
        nc,
        maps,
        core_ids=list(range(NCORES)),
        trace=trace,
        trace_cores=trace_cores,
    )
    total = 0.0
    for core_out in res.results:
        total += float(core_out["out"].astype(np.float64).sum())
    return np.float32(total), res


def kernel(repr, GT):
    total, _ = run_device(repr, GT, trace=False)
    return total


# revision 6
# speedup vs baseline: 1.2518x; 1.2518x over previous
"""Trainium2 Bass kernel for ClipPairWiseLossAll.

loss = sum_{i<j} || relu(r_i - r_j) ||_2   with r = repr[GT], M=512, N=768.

Strategy (8 NeuronCores, SPMD, single NEFF):
  * Host: gather r = repr[GT], transpose -> rT [N=768, M=512], cast bf16.
  * The pair space is indexed (t, s) with t < s; the "s" axis (512 values) is
    split across the 8 cores with a folded assignment (core c gets s in
    {16k+c, 16k+15-c}) so every core covers exactly 16352 pairs.
  * On device, rT lives in SBUF as 6 chunks of [128 n-partitions, 512 i-free].
    For each s: E[n, t] = relu(r[t,n] - r[s,n]) is ONE tensor_scalar per chunk
    (per-partition scalar = column rT[:, s]; runs in DVE 4x bf16 mode).
    E2 = E*E (DVE scalar_tensor_tensor or ACT Square, split to balance).
    A ones-column matmul contracts E2 over the n-partitions into PSUM row m_s
    (one-hot lhsT), accumulating the 6 chunks: psum[m_s, t] = sum_n E^2.
  * Valid-t ranges are rounded up to L_k = 16(k+1) (identical instruction
    shapes on every core -> one NEFF); a per-core 0/1 mask kills the invalid
    columns, then ACT computes sqrt with a fused row-sum. Host adds 8x64
    partial sums.
"""

import numpy as np

M = 512
N = 768
P = 128
NCH = N // P  # 6
NCORES = 8
NS = 64  # s values per core


def _s_list(c):
    out = []
    for k in range(32):
        out.append(16 * k + c)
        out.append(16 * k + 15 - c)
    return out


_PROG = {}

USE_FP8 = False


def SQUARE_ENGINE(k, slot):
    # tunable split of the E->E^2 pass across engines
    return "act"


def _build_program():
    if "nc" in _PROG:
        return _PROG["nc"]

    from contextlib import ExitStack

    import concourse.bacc as bacc
    import concourse.tile as tile
    from concourse import mybir

    AOT = mybir.AluOpType
    AFT = mybir.ActivationFunctionType
    bf16 = mybir.dt.bfloat16
    fp8 = mybir.dt.float8e4
    f32 = mybir.dt.float32

    nc = bacc.Bacc(
        "TRN2",
        target_bir_lowering=False,
        debug=False,
        enable_asserts=False,
        num_devices=NCORES,
    )

    rt_d = nc.dram_tensor("rt", [N, M], bf16, kind="ExternalInput")
    sc_d = nc.dram_tensor("sc", [N, NS], f32, kind="ExternalInput")
    mk_d = nc.dram_tensor("mk", [NS, M], f32, kind="ExternalInput")
    oh_d = nc.dram_tensor("oh", [P, NS * 2 * NS], USE_FP8 and fp8 or bf16, kind="ExternalInput")
    out_d = nc.dram_tensor("out", [NS, 1], f32, kind="ExternalOutput")

    with ExitStack() as ctx:
        tc = ctx.enter_context(tile.TileContext(nc))
        singles = ctx.enter_context(tc.tile_pool(name="singles", bufs=1))
        epool = ctx.enter_context(tc.tile_pool(name="e", bufs=4))
        e2pool = ctx.enter_context(tc.tile_pool(name="e2", bufs=4))
        pspool = ctx.enter_context(tc.tile_pool(name="ps", bufs=1, space="PSUM"))

        rt_sb = singles.tile([P, NCH, M], bf16)
        nc.sync.dma_start(out=rt_sb, in_=rt_d.ap().rearrange("(c p) i -> p c i", p=P))
        sc_sb = singles.tile([P, NCH, NS], f32)
        nc.sync.dma_start(out=sc_sb, in_=sc_d.ap().rearrange("(c p) s -> p c s", p=P))
        mk_sb = singles.tile([NS, M], f32)
        nc.sync.dma_start(out=mk_sb, in_=mk_d.ap())

        # fp8 one-hot lhsT stack (host-built, aligned): oh[:, m, :, :] is a
        # [128, 2, 64] dual-row lhsT whose ones-column lands psum row m.
        if USE_FP8:
            oh = singles.tile([P, NS, 2, NS], fp8)
            nc.sync.dma_start(out=oh, in_=oh_d.ap())
        else:
            ohb = singles.tile([P, NS, 1, NS], bf16)
            nc.sync.dma_start(
                out=ohb, in_=oh_d.ap().rearrange("p (m k c) -> p m k c", m=NS, k=2)[:, :, 0:1, :]
            )

        ps = pspool.tile([NS, M], f32)
        nc.vector.memset(ps, 0.0)

        for k in range(32):
            L = 16 * (k + 1)
            for slot in range(2):
                m = 2 * k + slot
                e_t = epool.tile([P, NCH, M], bf16, tag="e")
                for c in range(NCH):
                    nc.vector.tensor_scalar(
                        out=e_t[:, c, 0:L],
                        in0=rt_sb[:, c, 0:L],
                        scalar1=sc_sb[:, c, m : m + 1],
                        scalar2=0.0,
                        op0=AOT.subtract,
                        op1=AOT.max,
                    )
                e2_t = e2pool.tile([P, NCH, M], USE_FP8 and fp8 or bf16, tag="e2")
                eng = SQUARE_ENGINE(k, slot)
                if eng == "act":
                    nc.scalar.activation(
                        out=e2_t[:, :, 0:L],
                        in_=e_t[:, :, 0:L],
                        func=AFT.Square,
                    )
                elif eng == "gps":
                    nc.gpsimd.tensor_mul(
                        e2_t[:, :, 0:L], e_t[:, :, 0:L], e_t[:, :, 0:L]
                    )
                else:
                    nc.vector.tensor_mul(
                        e2_t[:, :, 0:L], e_t[:, :, 0:L], e_t[:, :, 0:L]
                    )
                if USE_FP8:
                    for c2 in range(NCH // 2):
                        nc.tensor.matmul(
                            ps[:, 0:L],
                            oh[:, m, :, :],
                            e2_t[:, 2 * c2 : 2 * c2 + 2, 0:L],
                            start=False,
                            stop=False,
                            skip_group_check=True,
                            perf_mode=mybir.MatmulPerfMode.DoubleRow,
                        )
                else:
                    for c in range(NCH):
                        nc.tensor.matmul(
                            ps[:, 0:L],
                            ohb[:, m, 0, :],
                            e2_t[:, c, 0:L],
                            start=False,
                            stop=False,
                            skip_group_check=True,
                        )

        masked = singles.tile([NS, M], f32)
        nc.vector.tensor_mul(masked, ps[:, :], mk_sb)
        sqrt_t = singles.tile([NS, M], bf16)
        res = singles.tile([NS, 1], f32)
        nc.scalar.activation(out=sqrt_t, in_=masked, func=AFT.Sqrt, accum_out=res)
        nc.sync.dma_start(out=out_d.ap(), in_=res)

    nc.compile()
    _PROG["nc"] = nc
    return nc


def _in_maps(repr_np, GT_np):
    import ml_dtypes

    r = np.asarray(repr_np, dtype=np.float32)[np.asarray(GT_np).astype(np.int64)]
    rT = np.ascontiguousarray(r.T)  # [N, M] f32
    rT_bf = rT.astype(ml_dtypes.bfloat16)

    ohdt = ml_dtypes.float8_e4m3 if USE_FP8 else ml_dtypes.bfloat16
    ohs = np.zeros((P, NS, 2, NS), dtype=ohdt)
    for m in range(NS):
        ohs[:, m, :, m] = 1.0
    ohs = ohs.reshape(P, NS * 2 * NS)

    maps = []
    t_idx = np.arange(M)[None, :]
    for c in range(NCORES):
        s = np.array(_s_list(c))
        sc = np.ascontiguousarray(rT_bf[:, s].astype(np.float32))  # [N, 64]
        mk = (t_idx < s[:, None]).astype(np.float32)  # [64, M]
        maps.append({"rt": rT_bf, "sc": sc, "mk": mk, "oh": ohs})
    return maps


def run_device(repr_np, GT_np, trace=False, trace_cores=None):
    """Run the bass kernel on 8 cores; returns (total, BassKernelResults)."""
    from concourse.bass_utils import run_bass_kernel_spmd

    nc = _build_program()
    maps = _in_maps(repr_np, GT_np)
    res = run_bass_kernel_spmd(
        nc,
        maps,
        core_ids=list(range(NCORES)),
        trace=trace,
        trace_cores=trace_cores,
    )
    total = 0.0
    for core_out in res.results:
        total += float(core_out["out"].astype(np.float64).sum())
    return np.float32(total), res


def kernel(repr, GT):
    total, _ = run_device(repr, GT, trace=False)
    return total


# revision 7
# speedup vs baseline: 1.2567x; 1.0040x over previous
"""Trainium2 Bass kernel for ClipPairWiseLossAll.

loss = sum_{i<j} || relu(r_i - r_j) ||_2   with r = repr[GT], M=512, N=768.

Strategy (8 NeuronCores, SPMD, single NEFF):
  * Host: gather r = repr[GT], transpose -> rT [N=768, M=512], cast bf16.
  * The pair space is indexed (t, s) with t < s; the "s" axis (512 values) is
    split across the 8 cores with a folded assignment (core c gets s in
    {16k+c, 16k+15-c}) so every core covers exactly 16352 pairs.
  * On device, rT lives in SBUF as 6 chunks of [128 n-partitions, 512 i-free].
    For each s: E[n, t] = relu(r[t,n] - r[s,n]) is ONE tensor_scalar per chunk
    (per-partition scalar = column rT[:, s]; runs in DVE 4x bf16 mode).
    E2 = E*E (DVE scalar_tensor_tensor or ACT Square, split to balance).
    A ones-column matmul contracts E2 over the n-partitions into PSUM row m_s
    (one-hot lhsT), accumulating the 6 chunks: psum[m_s, t] = sum_n E^2.
  * Valid-t ranges are rounded up to L_k = 16(k+1) (identical instruction
    shapes on every core -> one NEFF); a per-core 0/1 mask kills the invalid
    columns, then ACT computes sqrt with a fused row-sum. Host adds 8x64
    partial sums.
"""

import numpy as np

M = 512
N = 768
P = 128
NCH = N // P  # 6
NCORES = 8
NS = 64  # s values per core


def _s_list(c):
    out = []
    for k in range(32):
        out.append(16 * k + c)
        out.append(16 * k + 15 - c)
    return out


_PROG = {}

USE_FP8 = True


def SQUARE_ENGINE(k, slot):
    # tunable split of the E->E^2 pass across engines
    return "act"


def _build_program():
    if "nc" in _PROG:
        return _PROG["nc"]

    from contextlib import ExitStack

    import concourse.bacc as bacc
    import concourse.tile as tile
    from concourse import mybir

    AOT = mybir.AluOpType
    AFT = mybir.ActivationFunctionType
    bf16 = mybir.dt.bfloat16
    fp8 = mybir.dt.float8e4
    f32 = mybir.dt.float32

    nc = bacc.Bacc(
        "TRN2",
        target_bir_lowering=False,
        debug=False,
        enable_asserts=False,
        num_devices=NCORES,
    )

    rt_d = nc.dram_tensor("rt", [N, M], bf16, kind="ExternalInput")
    sc_d = nc.dram_tensor("sc", [N, NS], f32, kind="ExternalInput")
    mk_d = nc.dram_tensor("mk", [NS, M], f32, kind="ExternalInput")
    oh_d = nc.dram_tensor("oh", [P, NS * 2 * NS], USE_FP8 and fp8 or bf16, kind="ExternalInput")
    out_d = nc.dram_tensor("out", [NS, 1], f32, kind="ExternalOutput")

    with ExitStack() as ctx:
        tc = ctx.enter_context(tile.TileContext(nc))
        singles = ctx.enter_context(tc.tile_pool(name="singles", bufs=1))
        epool = ctx.enter_context(tc.tile_pool(name="e", bufs=4))
        e2pool = ctx.enter_context(tc.tile_pool(name="e2", bufs=4))
        pspool = ctx.enter_context(tc.tile_pool(name="ps", bufs=1, space="PSUM"))

        rt_sb = singles.tile([P, NCH, M], bf16)
        nc.sync.dma_start(out=rt_sb, in_=rt_d.ap().rearrange("(c p) i -> p c i", p=P))
        sc_sb = singles.tile([P, NCH, NS], f32)
        nc.sync.dma_start(out=sc_sb, in_=sc_d.ap().rearrange("(c p) s -> p c s", p=P))
        mk_sb = singles.tile([NS, M], f32)
        nc.sync.dma_start(out=mk_sb, in_=mk_d.ap())

        # fp8 one-hot lhsT stack (host-built, aligned): oh[:, m, :, :] is a
        # [128, 2, 64] dual-row lhsT whose ones-column lands psum row m.
        if USE_FP8:
            oh = singles.tile([P, NS, 2, NS], fp8)
            nc.sync.dma_start(out=oh, in_=oh_d.ap())
        else:
            ohb = singles.tile([P, NS, 1, NS], bf16)
            nc.sync.dma_start(
                out=ohb, in_=oh_d.ap().rearrange("p (m k c) -> p m k c", m=NS, k=2)[:, :, 0:1, :]
            )

        ps = pspool.tile([NS, M], f32)
        nc.vector.memset(ps, 0.0)

        for k in range(32):
            L = 16 * (k + 1)
            for slot in range(2):
                m = 2 * k + slot
                e_t = epool.tile([P, NCH, M], bf16, tag="e")
                for c in range(NCH):
                    nc.vector.tensor_scalar(
                        out=e_t[:, c, 0:L],
                        in0=rt_sb[:, c, 0:L],
                        scalar1=sc_sb[:, c, m : m + 1],
                        scalar2=0.0,
                        op0=AOT.subtract,
                        op1=AOT.max,
                    )
                e2_t = e2pool.tile([P, NCH, M], USE_FP8 and fp8 or bf16, tag="e2")
                eng = SQUARE_ENGINE(k, slot)
                if eng == "act":
                    nc.scalar.activation(
                        out=e2_t[:, :, 0:L],
                        in_=e_t[:, :, 0:L],
                        func=AFT.Square,
                    )
                elif eng == "gps":
                    nc.gpsimd.tensor_mul(
                        e2_t[:, :, 0:L], e_t[:, :, 0:L], e_t[:, :, 0:L]
                    )
                else:
                    nc.vector.tensor_mul(
                        e2_t[:, :, 0:L], e_t[:, :, 0:L], e_t[:, :, 0:L]
                    )
                if USE_FP8:
                    for c2 in range(NCH // 2):
                        nc.tensor.matmul(
                            ps[:, 0:L],
                            oh[:, m, :, :],
                            e2_t[:, 2 * c2 : 2 * c2 + 2, 0:L],
                            start=False,
                            stop=False,
                            skip_group_check=True,
                            perf_mode=mybir.MatmulPerfMode.DoubleRow,
                        )
                else:
                    for c in range(NCH):
                        nc.tensor.matmul(
                            ps[:, 0:L],
                            ohb[:, m, 0, :],
                            e2_t[:, c, 0:L],
                            start=False,
                            stop=False,
                            skip_group_check=True,
                        )

        masked = singles.tile([NS, M], f32)
        nc.vector.tensor_mul(masked, ps[:, :], mk_sb)
        sqrt_t = singles.tile([NS, M], bf16)
        res = singles.tile([NS, 1], f32)
        nc.scalar.activation(out=sqrt_t, in_=masked, func=AFT.Sqrt, accum_out=res)
        nc.sync.dma_start(out=out_d.ap(), in_=res)

    nc.compile()
    _PROG["nc"] = nc
    return nc


def _in_maps(repr_np, GT_np):
    import ml_dtypes

    r = np.asarray(repr_np, dtype=np.float32)[np.asarray(GT_np).astype(np.int64)]
    rT = np.ascontiguousarray(r.T)  # [N, M] f32
    rT_bf = rT.astype(ml_dtypes.bfloat16)

    ohdt = ml_dtypes.float8_e4m3 if USE_FP8 else ml_dtypes.bfloat16
    ohs = np.zeros((P, NS, 2, NS), dtype=ohdt)
    for m in range(NS):
        ohs[:, m, :, m] = 1.0
    ohs = ohs.reshape(P, NS * 2 * NS)

    maps = []
    t_idx = np.arange(M)[None, :]
    for c in range(NCORES):
        s = np.array(_s_list(c))
        sc = np.ascontiguousarray(rT_bf[:, s].astype(np.float32))  # [N, 64]
        mk = (t_idx < s[:, None]).astype(np.float32)  # [64, M]
        maps.append({"rt": rT_bf, "sc": sc, "mk": mk, "oh": ohs})
    return maps


def run_device(repr_np, GT_np, trace=False, trace_cores=None):
    """Run the bass kernel on 8 cores; returns (total, BassKernelResults)."""
    from concourse.bass_utils import run_bass_kernel_spmd

    nc = _build_program()
    maps = _in_maps(repr_np, GT_np)
    res = run_bass_kernel_spmd(
        nc,
        maps,
        core_ids=list(range(NCORES)),
        trace=trace,
        trace_cores=trace_cores,
    )
    total = 0.0
    for core_out in res.results:
        total += float(core_out["out"].astype(np.float64).sum())
    return np.float32(total), res


def kernel(repr, GT):
    total, _ = run_device(repr, GT, trace=False)
    return total
